# revision 7
# baseline (speedup 1.0000x reference)
"""Causal self-attention (RoPE) Trainium2 Bass kernel, v2.

Problem: B=4, T=2048, C=1024, H=16 heads, D=64, fp32 I/O.
Sharding: 8 cores = 4 (batch) x 2 (head-group TP). Each core computes
qkv/attention/proj for 1 batch and 8 heads, producing a partial
projection output; the host sums the two TP partials per batch.

v2 changes over baseline:
- QK score matmuls (K=64) emitted half-major so the two heads' MMs land
  on PE row-tiles (0,0)/(64,0) back-to-back and run concurrently.
- RoPE: ACT-engine PSUM evacuation, DMA cross-partition rotate-half swap,
  three fast SBUF tensor_tensor ops (DVE off the PSUM path).
- Causal ioff shrink applied to chunk 0 as well.
- One exp ACTIVATE per head per unit when io0==0 (the short unwritten gap
  is never consumed); two calls otherwise.
- Causal masks multiply only the 128-wide diagonal wedges with a shared
  [128,128] lower-triangle constant.
- Softmax reciprocal reads the PSUM denominator row directly.
- Stage-B PV matmuls lag one unit behind QK+exp (software pipeline).
"""

import numpy as np
from contextlib import ExitStack

import concourse.bacc as bacc
import concourse.bass as bass
import concourse.mybir as mybir
import concourse.tile as tile

# ---------------- constants ----------------
B = 4
T = 2048
C = 1024
H = 16
D = 64
L = 8  # local heads per core
NCORES = 8
ROPE_BASE = 10000.0

CH = 512  # t-chunk size
NCH = T // CH  # 4 chunks
KT = C // 128  # 8 contraction tiles
NP = L // 2  # 4 head-pair tiles
SCALE = 1.0 / np.sqrt(D)

F32 = mybir.dt.float32
BF16 = mybir.dt.bfloat16

DT_X = BF16
DT_K = BF16
DT_PV = BF16
DT_O = BF16


def _np_dt(dt):
    return mybir.dt.np(dt)


MM_KINDS = []


def _mm_kind(k):
    MM_KINDS.append(k)


# ---------------- device kernel ----------------


def attn_body(ctx: ExitStack, tc: tile.TileContext, outs, ins):
    """outs = (y [T, C] f32,); ins = (xt4, wqk, wv, wp, cs4, sn4, perm, wedge)."""
    nc = tc.nc
    (y,) = outs if isinstance(outs, (tuple, list)) else (outs,)
    xt4, wqk, wv, wp, cs4, sn4, wedge = ins

    consts = ctx.enter_context(tc.tile_pool(name="consts", bufs=1))
    xpool = ctx.enter_context(tc.tile_pool(name="xpool", bufs=16))
    cspool = ctx.enter_context(tc.tile_pool(name="cspool", bufs=4))
    qrpool = ctx.enter_context(tc.tile_pool(name="qrpool", bufs=8))
    rtmp = ctx.enter_context(tc.tile_pool(name="rtmp", bufs=6))
    ptpool = ctx.enter_context(tc.tile_pool(name="ptpool", bufs=8))
    otpool = ctx.enter_context(tc.tile_pool(name="otpool", bufs=16))
    yepool = ctx.enter_context(tc.tile_pool(name="yepool", bufs=3))
    lpool = ctx.enter_context(tc.tile_pool(name="lpool", bufs=3))
    pmisc = ctx.enter_context(tc.tile_pool(name="pmisc", bufs=2, space="PSUM"))
    pss_pool = ctx.enter_context(tc.tile_pool(name="pss", bufs=2, space="PSUM"))
    pso_pool = ctx.enter_context(tc.tile_pool(name="pso", bufs=2, space="PSUM"))

    # persistent tiles
    wqk_sb = [consts.tile([128, 2 * 512], DT_X, name=f"wqk{k}") for k in range(KT)]
    wv_sb = [consts.tile([128, 512], DT_X, name=f"wv{k}") for k in range(KT)]
    wp_sb = [consts.tile([128, C], DT_O, name=f"wp{p}") for p in range(NP)]
    wedge_sb = consts.tile([128, 128], DT_PV, name="wedge")
    k_rot = [consts.tile([128, T], DT_K, name=f"krot{p}") for p in range(NP)]
    v_sb = consts.tile([128, T // 128, L, 65], DT_PV, name="vsb")

    def warmup_pe():
        # ~5.5us of dummy matmuls on a zeroed tile so the PE HAM clock-gate
        # is warm (2.4 GHz) by the time real matmuls issue, overlapping the
        # initial DMA window.
        warm = consts.tile([128, CH], DT_X, name="warm")
        nc.vector.memset(warm[:], 0.0)
        wps = pmisc.tile([128, CH], F32, name="warmps", tag="pA")
        for i in range(30):
            _mm_kind(("warm", CH))
            nc.tensor.matmul(wps[:], warm[:, 0:128], warm[:],
                             start=(i == 0), stop=(i == 29))

    def load_first_chunk():
        xt_sb[0] = []
        for k in range(KT):
            nc.sync.dma_start(wqk_sb[k][:], wqk[k])
            xt = xpool.tile([128, CH], DT_X, name=f"xt0_{k}", tag="xt")
            nc.sync.dma_start(xt[:], xt4[0, k])
            xt_sb[0].append(xt)
        cs_sb[0] = cspool.tile([128, CH], DT_K, name="cs0", tag="cs")
        sn_sb[0] = cspool.tile([128, CH], DT_K, name="sn0", tag="sn")
        nc.sync.dma_start(cs_sb[0][:], cs4[0])
        nc.sync.dma_start(sn_sb[0][:], sn4[0])
        for k in range(KT):
            nc.sync.dma_start(wv_sb[k][:], wv[k])
        nc.vector.memset(v_sb[:, :, :, 64:65], 1.0)

    def load_consts_late():
        nc.sync.dma_start(wedge_sb[:], wedge[:])
        for p in range(NP):
            nc.sync.dma_start(wp_sb[p][:], wp[p])

    # per-chunk transient state
    xt_sb = {}
    q_rot = {}
    cs_sb = {}
    sn_sb = {}
    ot_sb = {}

    def load_chunk_inputs(c):
        def f():
            cs_sb[c] = cspool.tile([128, CH], DT_K, name=f"cs{c}", tag="cs")
            sn_sb[c] = cspool.tile([128, CH], DT_K, name=f"sn{c}", tag="sn")
            nc.sync.dma_start(cs_sb[c][:], cs4[c])
            nc.sync.dma_start(sn_sb[c][:], sn4[c])
            xt_sb[c] = []
            for k in range(KT):
                xt = xpool.tile([128, CH], DT_X, name=f"xt{c}_{k}", tag="xt")
                nc.sync.dma_start(xt[:], xt4[c, k])
                xt_sb[c].append(xt)

        return [f]

    def stage_a_units(c):
        """12 units: 8 q/k feature tiles + 4 v t-blocks for chunk c.

        RoPE per q/k tile: ACT evacuates PSUM to bf16 (freeing the PSUM slot
        off the DVE path), DMA performs the cross-partition rotate-half swap,
        and three fast SBUF tensor_tensor ops finish rot = q*cos + swap*sin.
        """

        def mk_qk(c, jt):
            def f():
                ps = pmisc.tile([128, CH], F32, name=f"psA{c}_{jt}", tag="pA")
                for k in range(KT):
                    _mm_kind(("aqk", CH))
                    nc.tensor.matmul(
                        ps[:],
                        wqk_sb[k][:, jt * 128 : (jt + 1) * 128],
                        xt_sb[c][k][:],
                        start=(k == 0),
                        stop=(k == KT - 1),
                    )
                q_sb = rtmp.tile([128, CH], DT_K, name=f"qsb{c}_{jt}", tag="qsb")
                if jt < NP:
                    nc.scalar.activation(
                        q_sb[:], ps[:], mybir.ActivationFunctionType.Copy
                    )
                else:
                    nc.vector.tensor_copy(q_sb[:], ps[:])
                qsw = rtmp.tile([128, CH], DT_K, name=f"qsw{c}_{jt}", tag="qsw")
                for blk in range(2):
                    b0 = blk * 64
                    nc.sync.dma_start(
                        qsw[b0 : b0 + 32, :], q_sb[b0 + 32 : b0 + 64, :]
                    )
                    nc.sync.dma_start(
                        qsw[b0 + 32 : b0 + 64, :], q_sb[b0 : b0 + 32, :]
                    )
                qtmp = rtmp.tile([128, CH], DT_K, name=f"qtmp{c}_{jt}", tag="qtmp")
                nc.vector.tensor_tensor(
                    out=qtmp[:], in0=qsw[:], in1=sn_sb[c][:],
                    op=mybir.AluOpType.mult,
                )
                qraw = rtmp.tile([128, CH], DT_K, name=f"qraw{c}_{jt}", tag="qraw")
                nc.vector.tensor_tensor(
                    out=qraw[:], in0=q_sb[:], in1=cs_sb[c][:],
                    op=mybir.AluOpType.mult,
                )
                if jt < NP:
                    dst_t = qrpool.tile([128, CH], DT_K, name=f"qrot{c}_{jt}", tag="qr")
                    q_rot[(c, jt)] = dst_t
                    out_ap = dst_t[:]
                else:
                    out_ap = k_rot[jt - NP][:, c * CH : (c + 1) * CH]
                nc.vector.tensor_tensor(
                    out=out_ap, in0=qraw[:], in1=qtmp[:], op=mybir.AluOpType.add
                )

            return f

        def mk_v(c, tbl):
            tb = c * 4 + tbl

            def f():
                ps = pmisc.tile([128, CH], F32, name=f"psV{c}_{tbl}", tag="pA")
                for k in range(KT):
                    _mm_kind(("av", CH))
                    nc.tensor.matmul(
                        ps[:],
                        xt_sb[c][k][:, tbl * 128 : (tbl + 1) * 128],
                        wv_sb[k][:],
                        start=(k == 0),
                        stop=(k == KT - 1),
                    )
                nc.vector.tensor_copy(
                    v_sb[:, tb, :, 0:64],
                    ps[:].rearrange("p (h d) -> p h d", h=L),
                )

            return f

        units = [mk_qk(c, jt) for jt in range(2 * NP)]
        units += [mk_v(c, tbl) for tbl in range(4)]
        return units

    def stage_b_units(c):
        """QK+exp+mask / PV micro-units with PV lagging one unit."""

        def _ioff(jb, jp):
            return (jb - 4 * c) * 128 if jp >= 2 * c else 0

        pss_unit = {}
        pt_unit = {}
        pso_unit = {}

        def mk_qk_exp(p, jp):
            def f():
                ha, hb = 2 * p, 2 * p + 1
                qt = q_rot[(c, p)]
                kt_ = k_rot[p]
                for h in (ha, hb):
                    pss_unit[(h, jp)] = pss_pool.tile(
                        [128, 2 * CH], F32, name=f"pss{c}_{p}_{jp}_{h}", tag="pss"
                    )
                # 64-row-mode score matmuls (K=64). Half-array activity keeps
                # 8-core package power below the P0 downclock threshold --
                # full-array variants measurably drop the whole chip to 2 GHz.
                for half in range(2):
                    jb = 2 * jp + half
                    io = _ioff(jb, jp)
                    for h, rb in ((ha, 0), (hb, 64)):
                        _mm_kind(("qk", CH - io))
                        nc.tensor.matmul(
                            pss_unit[(h, jp)][:, half * CH + io : (half + 1) * CH],
                            kt_[rb : rb + 64, jb * 128 : (jb + 1) * 128],
                            qt[rb : rb + 64, io:],
                            start=True,
                            stop=True,
                        )
                io0 = _ioff(2 * jp, jp)
                io1 = _ioff(2 * jp + 1, jp)
                for h in (ha, hb):
                    pt = ptpool.tile(
                        [128, 2 * CH], DT_PV, name=f"pt{c}_{p}_{jp}_{h}", tag="pt"
                    )
                    pt_unit[(h, jp)] = pt
                    if io0 == 0:
                        # one call covers [0, 2CH); the short unwritten gap
                        # before CH+io1 yields garbage that is never consumed
                        nc.scalar.activation(
                            pt[:],
                            pss_unit[(h, jp)][:],
                            mybir.ActivationFunctionType.Exp,
                            scale=float(SCALE),
                        )
                    else:
                        for lo, hi in ((io0, CH), (CH + io1, 2 * CH)):
                            nc.scalar.activation(
                                pt[:, lo:hi],
                                pss_unit[(h, jp)][:, lo:hi],
                                mybir.ActivationFunctionType.Exp,
                                scale=float(SCALE),
                            )
                    if jp >= 2 * c:  # diagonal: mask the two 128-wide wedges
                        for half in range(2):
                            jb = 2 * jp + half
                            io = _ioff(jb, jp)
                            lo = half * CH + io
                            nc.vector.tensor_tensor(
                                out=pt[:, lo : lo + 128],
                                in0=pt[:, lo : lo + 128],
                                in1=wedge_sb[:],
                                op=mybir.AluOpType.mult,
                            )

            return f

        def mk_pv(p, jp):
            def f():
                ha, hb = 2 * p, 2 * p + 1
                if jp == 0:
                    for h in (ha, hb):
                        pso_unit[h] = pso_pool.tile(
                            [65, CH], F32, name=f"pso{c}_{h}", tag="pso"
                        )
                for h in (ha, hb):
                    pt = pt_unit.pop((h, jp))
                    pso = pso_unit[h]
                    for half in range(2):
                        jb = 2 * jp + half
                        io = _ioff(jb, jp)
                        _mm_kind(("pv", CH - io))
                        nc.tensor.matmul(
                            pso[:, io:],
                            v_sb[:, jb, h, 0:65],
                            pt[:, half * CH + io : (half + 1) * CH],
                            start=(jp == 0 and half == 0),
                            stop=(jp == 2 * c + 1 and half == 1),
                        )

            return f

        def mk_norm(p):
            def f():
                ot = ot_sb[(c, p)] = otpool.tile(
                    [128, CH], DT_O, name=f"ot{c}_{p}", tag="ot"
                )
                for idx, h in enumerate((2 * p, 2 * p + 1)):
                    pso = pso_unit[h]
                    # reciprocal_approx_fast cannot read PSUM (custom DVE op);
                    # evacuate the denominator row via ScalarE first.
                    lsb = lpool.tile([1, CH], F32, name=f"lsb{c}_{h}", tag="lsb")
                    nc.vector.tensor_copy(lsb[:], pso[64:65, :])
                    linv = lpool.tile([1, CH], F32, name=f"linv{c}_{h}", tag="linv")
                    nc.vector.reciprocal_approx_fast(linv[:], lsb[:])
                    lb = lpool.tile([64, CH], F32, name=f"lb{c}_{h}", tag="lb")
                    nc.gpsimd.partition_broadcast(lb[:], linv[:])
                    nc.vector.tensor_tensor(
                        out=ot[idx * 64 : (idx + 1) * 64, :],
                        in0=pso[0:64, :],
                        in1=lb[:],
                        op=mybir.AluOpType.mult,
                    )

            return f

        units = []
        pending = []  # (pv_unit, norm_or_None)
        for p in range(NP):
            njp = 2 * c + 2
            for jp in range(njp):
                units.append(mk_qk_exp(p, jp))
                pending.append((mk_pv(p, jp), mk_norm(p) if jp == njp - 1 else None))
                if len(pending) > 1:
                    pv, nrm = pending.pop(0)
                    units.append(pv)
                    if nrm is not None:
                        units.append(nrm)
        while pending:
            pv, nrm = pending.pop(0)
            units.append(pv)
            if nrm is not None:
                units.append(nrm)
        return units

    def proj_units(c):
        units = []

        def mk_proj(c, tbl, oc):
            def f():
                ps = pmisc.tile([128, CH], F32, name=f"psY{c}_{tbl}_{oc}", tag="pA")
                for p in range(NP):
                    _mm_kind(("proj", CH))
                    nc.tensor.matmul(
                        ps[:],
                        ot_sb[(c, p)][:, tbl * 128 : (tbl + 1) * 128],
                        wp_sb[p][:, oc * CH : (oc + 1) * CH],
                        start=(p == 0),
                        stop=(p == NP - 1),
                    )
                ye = yepool.tile([128, CH], DT_O, name=f"ye{c}_{tbl}_{oc}", tag="ye")
                nc.vector.tensor_copy(ye[:], ps[:])
                nc.sync.dma_start(
                    y[c * CH + tbl * 128 : c * CH + (tbl + 1) * 128,
                      oc * CH : (oc + 1) * CH],
                    ye[:],
                )

            return f

        for tbl in range(4):
            for oc in range(C // CH):
                units.append(mk_proj(c, tbl, oc))
        return units

    def emit_interleaved(primary, secondary):
        if not primary:
            for u in secondary:
                u()
            return
        ns, npri = len(secondary), len(primary)
        si = 0
        for i, u in enumerate(primary):
            u()
            want = (i + 1) * ns // npri
            while si < want:
                secondary[si]()
                si += 1

    # ---- emission ----
    warmup_pe()
    load_first_chunk()
    for u in stage_a_units(0):
        u()
    load_consts_late()
    for c in range(NCH):
        fill = []
        if c + 1 < NCH:
            fill += load_chunk_inputs(c + 1)
            fill += stage_a_units(c + 1)
        if c == NCH - 1:
            for cc in range(NCH - 1):
                fill += proj_units(cc)
        emit_interleaved(stage_b_units(c), fill)
    for u in proj_units(NCH - 1):
        u()


def build_nc():
    nc = bacc.Bacc("TRN2", target_bir_lowering=False, debug=False)
    xt4 = nc.declare_dram_parameter("xt4", [NCH, KT, 128, CH], DT_X, isOutput=False)
    wqk = nc.declare_dram_parameter("wqk", [KT, 128, 1024], DT_X, isOutput=False)
    wv = nc.declare_dram_parameter("wv", [KT, 128, 512], DT_X, isOutput=False)
    wp = nc.declare_dram_parameter("wp", [NP, 128, C], DT_O, isOutput=False)
    cs4 = nc.declare_dram_parameter("cs4", [NCH, 128, CH], DT_K, isOutput=False)
    sn4 = nc.declare_dram_parameter("sn4", [NCH, 128, CH], DT_K, isOutput=False)
    wedge = nc.declare_dram_parameter("wedge", [128, 128], DT_PV, isOutput=False)
    yout = nc.declare_dram_parameter("y", [T, C], BF16, isOutput=True)

    with tile.TileContext(nc) as tc:
        with ExitStack() as ctx:
            attn_body(
                ctx, tc, (yout[:],),
                (xt4[:], wqk[:], wv[:], wp[:], cs4[:], sn4[:], wedge[:]),
            )
    nc.compile()
    return nc


# ---------------- host side ----------------


def _rope_tables_np():
    inv_freq = 1.0 / (ROPE_BASE ** (np.arange(0, D, 2, dtype=np.float64) / D))
    t = np.arange(T, dtype=np.float64)
    freqs = np.outer(t, inv_freq)  # [T, 32]
    emb = np.concatenate([freqs, freqs], axis=-1)  # [T, 64]
    return np.cos(emb), np.sin(emb)  # [T, 64] each


def _host_tables():
    cos, sin = _rope_tables_np()  # [T, 64]
    d_of_r = np.arange(128) % 64
    cs = cos[:, d_of_r].T.astype(np.float32)  # [128, T]
    sn_abs = sin[:, d_of_r].T
    sign = np.where((d_of_r % 64) < 32, -1.0, 1.0)[:, None]
    sn = (sn_abs * sign).astype(np.float32)  # [128, T]
    np_k = _np_dt(DT_K)
    cs4 = np.ascontiguousarray(cs.reshape(128, NCH, CH).transpose(1, 0, 2)).astype(np_k)
    sn4 = np.ascontiguousarray(sn.reshape(128, NCH, CH).transpose(1, 0, 2)).astype(np_k)

    jj = np.arange(128)[:, None]
    ii = np.arange(128)[None, :]
    wedge = (jj <= ii).astype(np.float64)
    return cs4, sn4, wedge


def make_core_inputs(x, Wqkv, Wproj, core):
    b, g = core // 2, core % 2
    np_x = _np_dt(DT_X)
    np_pv = _np_dt(DT_PV)
    np_o = _np_dt(DT_O)

    xT = np.ascontiguousarray(x[b].T)  # [C, T]
    xt4 = np.ascontiguousarray(
        xT.reshape(KT, 128, NCH, CH).transpose(2, 0, 1, 3)
    ).astype(np_x)

    Wq = Wqkv[g * 512 : (g + 1) * 512]
    Wk = Wqkv[C + g * 512 : C + (g + 1) * 512]
    Wv = Wqkv[2 * C + g * 512 : 2 * C + (g + 1) * 512]
    wqkT = np.vstack([Wq, Wk]).T  # [C, 1024]
    wqk = np.ascontiguousarray(wqkT.reshape(KT, 128, 1024)).astype(np_x)
    wvT = Wv.T  # [C, 512]
    wv = np.ascontiguousarray(wvT.reshape(KT, 128, 512)).astype(np_x)
    wpT = Wproj[:, g * 512 : (g + 1) * 512].T  # [512, C]
    wp = np.ascontiguousarray(wpT.reshape(NP, 128, C)).astype(np_o)

    cs4, sn4, wedge = _host_tables()
    return {
        "xt4": xt4,
        "wqk": wqk,
        "wv": wv,
        "wp": wp,
        "cs4": cs4,
        "sn4": sn4,
        "wedge": wedge.astype(np_pv),
    }


LAST_RESULTS = None
_NC_CACHE = None


def kernel(x, Wqkv, Wproj):
    global LAST_RESULTS, _NC_CACHE
    from concourse.bass_utils import run_bass_kernel_spmd

    x = np.asarray(x, dtype=np.float32)
    Wqkv = np.asarray(Wqkv, dtype=np.float32)
    Wproj = np.asarray(Wproj, dtype=np.float32)

    if _NC_CACHE is None:
        _NC_CACHE = build_nc()
    nc = _NC_CACHE
    in_maps = [make_core_inputs(x, Wqkv, Wproj, core) for core in range(NCORES)]
    res = run_bass_kernel_spmd(nc, in_maps, list(range(NCORES)))
    LAST_RESULTS = res

    out = np.empty((B, T, C), dtype=np.float32)
    for b in range(B):
        out[b] = (res.results[2 * b]["y"].astype(np.float32)
                  + res.results[2 * b + 1]["y"].astype(np.float32))
    return out


# revision 8
# speedup vs baseline: 1.0009x; 1.0009x over previous
"""Causal self-attention (RoPE) Trainium2 Bass kernel, v2.

Problem: B=4, T=2048, C=1024, H=16 heads, D=64, fp32 I/O.
Sharding: 8 cores = 4 (batch) x 2 (head-group TP). Each core computes
qkv/attention/proj for 1 batch and 8 heads, producing a partial
projection output; the host sums the two TP partials per batch.

v2 changes over baseline:
- QK score matmuls (K=64) emitted half-major so the two heads' MMs land
  on PE row-tiles (0,0)/(64,0) back-to-back and run concurrently.
- RoPE: ACT-engine PSUM evacuation, DMA cross-partition rotate-half swap,
  three fast SBUF tensor_tensor ops (DVE off the PSUM path).
- Causal ioff shrink applied to chunk 0 as well.
- One exp ACTIVATE per head per unit when io0==0 (the short unwritten gap
  is never consumed); two calls otherwise.
- Causal masks multiply only the 128-wide diagonal wedges with a shared
  [128,128] lower-triangle constant.
- Softmax reciprocal reads the PSUM denominator row directly.
- Stage-B PV matmuls lag one unit behind QK+exp (software pipeline).
"""

import numpy as np
from contextlib import ExitStack

import concourse.bacc as bacc
import concourse.bass as bass
import concourse.mybir as mybir
import concourse.tile as tile

# ---------------- constants ----------------
B = 4
T = 2048
C = 1024
H = 16
D = 64
L = 8  # local heads per core
NCORES = 8
ROPE_BASE = 10000.0

CH = 512  # t-chunk size
NCH = T // CH  # 4 chunks
KT = C // 128  # 8 contraction tiles
NP = L // 2  # 4 head-pair tiles
SCALE = 1.0 / np.sqrt(D)

F32 = mybir.dt.float32
BF16 = mybir.dt.bfloat16

DT_X = BF16
DT_K = BF16
DT_PV = BF16
DT_O = BF16


def _np_dt(dt):
    return mybir.dt.np(dt)


MM_KINDS = []


def _mm_kind(k):
    MM_KINDS.append(k)


# ---------------- device kernel ----------------


def attn_body(ctx: ExitStack, tc: tile.TileContext, outs, ins):
    """outs = (y [T, C] f32,); ins = (xt4, wqk, wv, wp, cs4, sn4, perm, wedge)."""
    nc = tc.nc
    (y,) = outs if isinstance(outs, (tuple, list)) else (outs,)
    xt4, wqk, wv, wp, cs4, sn4, wedge = ins

    consts = ctx.enter_context(tc.tile_pool(name="consts", bufs=1))
    xpool = ctx.enter_context(tc.tile_pool(name="xpool", bufs=16))
    cspool = ctx.enter_context(tc.tile_pool(name="cspool", bufs=4))
    qrpool = ctx.enter_context(tc.tile_pool(name="qrpool", bufs=8))
    rtmp = ctx.enter_context(tc.tile_pool(name="rtmp", bufs=6))
    ptpool = ctx.enter_context(tc.tile_pool(name="ptpool", bufs=6))
    otpool = ctx.enter_context(tc.tile_pool(name="otpool", bufs=16))
    yepool = ctx.enter_context(tc.tile_pool(name="yepool", bufs=3))
    lpool = ctx.enter_context(tc.tile_pool(name="lpool", bufs=3))
    pmisc = ctx.enter_context(tc.tile_pool(name="pmisc", bufs=2, space="PSUM"))
    pss_pool = ctx.enter_context(tc.tile_pool(name="pss", bufs=2, space="PSUM"))
    pso_pool = ctx.enter_context(tc.tile_pool(name="pso", bufs=2, space="PSUM"))

    # persistent tiles
    wqk_sb = [consts.tile([128, 2 * 512], DT_X, name=f"wqk{k}") for k in range(KT)]
    wv_sb = [consts.tile([128, 512], DT_X, name=f"wv{k}") for k in range(KT)]
    wp_sb = [consts.tile([128, C], DT_O, name=f"wp{p}") for p in range(NP)]
    wedge_sb = consts.tile([128, 128], DT_PV, name="wedge")
    k_rot = [consts.tile([128, T], DT_K, name=f"krot{p}") for p in range(NP)]
    v_sb = consts.tile([128, T // 128, L, 65], DT_PV, name="vsb")

    def warmup_pe():
        # ~5.5us of dummy matmuls on a zeroed tile so the PE HAM clock-gate
        # is warm (2.4 GHz) by the time real matmuls issue, overlapping the
        # initial DMA window.
        warm = consts.tile([128, CH], DT_X, name="warm")
        nc.vector.memset(warm[:], 0.0)
        wps = pmisc.tile([128, CH], F32, name="warmps", tag="pA")
        for i in range(26):
            _mm_kind(("warm", CH))
            nc.tensor.matmul(wps[:], warm[:, 0:128], warm[:],
                             start=(i == 0), stop=(i == 25))

    def load_first_chunk():
        xt_sb[0] = []
        for k in range(KT):
            nc.sync.dma_start(wqk_sb[k][:], wqk[k])
            xt = xpool.tile([128, CH], DT_X, name=f"xt0_{k}", tag="xt")
            nc.sync.dma_start(xt[:], xt4[0, k])
            xt_sb[0].append(xt)
        cs_sb[0] = cspool.tile([128, CH], DT_K, name="cs0", tag="cs")
        sn_sb[0] = cspool.tile([128, CH], DT_K, name="sn0", tag="sn")
        nc.sync.dma_start(cs_sb[0][:], cs4[0])
        nc.sync.dma_start(sn_sb[0][:], sn4[0])
        for k in range(KT):
            nc.sync.dma_start(wv_sb[k][:], wv[k])
        nc.vector.memset(v_sb[:, :, :, 64:65], 1.0)

    def load_consts_late():
        nc.sync.dma_start(wedge_sb[:], wedge[:])
        for p in range(NP):
            nc.sync.dma_start(wp_sb[p][:], wp[p])

    # per-chunk transient state
    xt_sb = {}
    q_rot = {}
    cs_sb = {}
    sn_sb = {}
    ot_sb = {}

    def load_chunk_inputs(c):
        def f():
            cs_sb[c] = cspool.tile([128, CH], DT_K, name=f"cs{c}", tag="cs")
            sn_sb[c] = cspool.tile([128, CH], DT_K, name=f"sn{c}", tag="sn")
            nc.sync.dma_start(cs_sb[c][:], cs4[c])
            nc.sync.dma_start(sn_sb[c][:], sn4[c])
            xt_sb[c] = []
            for k in range(KT):
                xt = xpool.tile([128, CH], DT_X, name=f"xt{c}_{k}", tag="xt")
                nc.sync.dma_start(xt[:], xt4[c, k])
                xt_sb[c].append(xt)

        return [f]

    def stage_a_units(c):
        """12 units: 8 q/k feature tiles + 4 v t-blocks for chunk c.

        RoPE per q/k tile: ACT evacuates PSUM to bf16 (freeing the PSUM slot
        off the DVE path), DMA performs the cross-partition rotate-half swap,
        and three fast SBUF tensor_tensor ops finish rot = q*cos + swap*sin.
        """

        def mk_qk(c, jt):
            def f():
                ps = pmisc.tile([128, CH], F32, name=f"psA{c}_{jt}", tag="pA")
                for k in range(KT):
                    _mm_kind(("aqk", CH))
                    nc.tensor.matmul(
                        ps[:],
                        wqk_sb[k][:, jt * 128 : (jt + 1) * 128],
                        xt_sb[c][k][:],
                        start=(k == 0),
                        stop=(k == KT - 1),
                    )
                q_sb = rtmp.tile([128, CH], DT_K, name=f"qsb{c}_{jt}", tag="qsb")
                nc.scalar.activation(
                    q_sb[:], ps[:], mybir.ActivationFunctionType.Copy
                )
                qsw = rtmp.tile([128, CH], DT_K, name=f"qsw{c}_{jt}", tag="qsw")
                for blk in range(2):
                    b0 = blk * 64
                    nc.sync.dma_start(
                        qsw[b0 : b0 + 32, :], q_sb[b0 + 32 : b0 + 64, :]
                    )
                    nc.sync.dma_start(
                        qsw[b0 + 32 : b0 + 64, :], q_sb[b0 : b0 + 32, :]
                    )
                qtmp = rtmp.tile([128, CH], DT_K, name=f"qtmp{c}_{jt}", tag="qtmp")
                nc.vector.tensor_tensor(
                    out=qtmp[:], in0=qsw[:], in1=sn_sb[c][:],
                    op=mybir.AluOpType.mult,
                )
                qraw = rtmp.tile([128, CH], DT_K, name=f"qraw{c}_{jt}", tag="qraw")
                nc.vector.tensor_tensor(
                    out=qraw[:], in0=q_sb[:], in1=cs_sb[c][:],
                    op=mybir.AluOpType.mult,
                )
                if jt < NP:
                    dst_t = qrpool.tile([128, CH], DT_K, name=f"qrot{c}_{jt}", tag="qr")
                    q_rot[(c, jt)] = dst_t
                    out_ap = dst_t[:]
                else:
                    out_ap = k_rot[jt - NP][:, c * CH : (c + 1) * CH]
                nc.vector.tensor_tensor(
                    out=out_ap, in0=qraw[:], in1=qtmp[:], op=mybir.AluOpType.add
                )

            return f

        def mk_v(c, tbl):
            tb = c * 4 + tbl

            def f():
                ps = pmisc.tile([128, CH], F32, name=f"psV{c}_{tbl}", tag="pA")
                for k in range(KT):
                    _mm_kind(("av", CH))
                    nc.tensor.matmul(
                        ps[:],
                        xt_sb[c][k][:, tbl * 128 : (tbl + 1) * 128],
                        wv_sb[k][:],
                        start=(k == 0),
                        stop=(k == KT - 1),
                    )
                nc.vector.tensor_copy(
                    v_sb[:, tb, :, 0:64],
                    ps[:].rearrange("p (h d) -> p h d", h=L),
                )

            return f

        units = [mk_qk(c, jt) for jt in range(2 * NP)]
        units += [mk_v(c, tbl) for tbl in range(4)]
        return units

    def stage_b_units(c):
        """QK+exp+mask / PV micro-units with PV lagging one unit."""

        def _ioff(jb, jp):
            return (jb - 4 * c) * 128 if jp >= 2 * c else 0

        pss_unit = {}
        pt_unit = {}
        pso_unit = {}

        def mk_qk_exp(p, jp):
            def f():
                ha, hb = 2 * p, 2 * p + 1
                qt = q_rot[(c, p)]
                kt_ = k_rot[p]
                for h in (ha, hb):
                    pss_unit[(h, jp)] = pss_pool.tile(
                        [128, 2 * CH], F32, name=f"pss{c}_{p}_{jp}_{h}", tag="pss"
                    )
                # 64-row-mode score matmuls (K=64). Half-array activity keeps
                # 8-core package power below the P0 downclock threshold --
                # full-array variants measurably drop the whole chip to 2 GHz.
                for half in range(2):
                    jb = 2 * jp + half
                    io = _ioff(jb, jp)
                    for h, rb in ((ha, 0), (hb, 64)):
                        _mm_kind(("qk", CH - io))
                        nc.tensor.matmul(
                            pss_unit[(h, jp)][:, half * CH + io : (half + 1) * CH],
                            kt_[rb : rb + 64, jb * 128 : (jb + 1) * 128],
                            qt[rb : rb + 64, io:],
                            start=True,
                            stop=True,
                        )
                io0 = _ioff(2 * jp, jp)
                io1 = _ioff(2 * jp + 1, jp)
                for h in (ha, hb):
                    pt = ptpool.tile(
                        [128, 2 * CH], DT_PV, name=f"pt{c}_{p}_{jp}_{h}", tag="pt"
                    )
                    pt_unit[(h, jp)] = pt
                    if io0 == 0:
                        # one call covers [0, 2CH); the short unwritten gap
                        # before CH+io1 yields garbage that is never consumed
                        nc.scalar.activation(
                            pt[:],
                            pss_unit[(h, jp)][:],
                            mybir.ActivationFunctionType.Exp,
                            scale=float(SCALE),
                        )
                    else:
                        for lo, hi in ((io0, CH), (CH + io1, 2 * CH)):
                            nc.scalar.activation(
                                pt[:, lo:hi],
                                pss_unit[(h, jp)][:, lo:hi],
                                mybir.ActivationFunctionType.Exp,
                                scale=float(SCALE),
                            )
                    if jp >= 2 * c:  # diagonal: mask the two 128-wide wedges
                        for half in range(2):
                            jb = 2 * jp + half
                            io = _ioff(jb, jp)
                            lo = half * CH + io
                            nc.vector.tensor_tensor(
                                out=pt[:, lo : lo + 128],
                                in0=pt[:, lo : lo + 128],
                                in1=wedge_sb[:],
                                op=mybir.AluOpType.mult,
                            )

            return f

        def mk_pv(p, jp):
            def f():
                ha, hb = 2 * p, 2 * p + 1
                if jp == 0:
                    for h in (ha, hb):
                        pso_unit[h] = pso_pool.tile(
                            [65, CH], F32, name=f"pso{c}_{h}", tag="pso"
                        )
                for h in (ha, hb):
                    pt = pt_unit.pop((h, jp))
                    pso = pso_unit[h]
                    for half in range(2):
                        jb = 2 * jp + half
                        io = _ioff(jb, jp)
                        _mm_kind(("pv", CH - io))
                        nc.tensor.matmul(
                            pso[:, io:],
                            v_sb[:, jb, h, 0:65],
                            pt[:, half * CH + io : (half + 1) * CH],
                            start=(jp == 0 and half == 0),
                            stop=(jp == 2 * c + 1 and half == 1),
                        )

            return f

        def mk_norm(p):
            def f():
                ot = ot_sb[(c, p)] = otpool.tile(
                    [128, CH], DT_O, name=f"ot{c}_{p}", tag="ot"
                )
                for idx, h in enumerate((2 * p, 2 * p + 1)):
                    pso = pso_unit[h]
                    # reciprocal_approx_fast cannot read PSUM (custom DVE op);
                    # evacuate the denominator row via ScalarE first.
                    lsb = lpool.tile([1, CH], F32, name=f"lsb{c}_{h}", tag="lsb")
                    nc.vector.tensor_copy(lsb[:], pso[64:65, :])
                    linv = lpool.tile([1, CH], F32, name=f"linv{c}_{h}", tag="linv")
                    nc.vector.reciprocal_approx_fast(linv[:], lsb[:])
                    lb = lpool.tile([64, CH], F32, name=f"lb{c}_{h}", tag="lb")
                    nc.gpsimd.partition_broadcast(lb[:], linv[:])
                    nc.vector.tensor_tensor(
                        out=ot[idx * 64 : (idx + 1) * 64, :],
                        in0=pso[0:64, :],
                        in1=lb[:],
                        op=mybir.AluOpType.mult,
                    )

            return f

        units = []
        pending = []  # (pv_unit, norm_or_None)
        for p in range(NP):
            njp = 2 * c + 2
            for jp in range(njp):
                units.append(mk_qk_exp(p, jp))
                pending.append((mk_pv(p, jp), mk_norm(p) if jp == njp - 1 else None))
                if len(pending) > 1:
                    pv, nrm = pending.pop(0)
                    units.append(pv)
                    if nrm is not None:
                        units.append(nrm)
        while pending:
            pv, nrm = pending.pop(0)
            units.append(pv)
            if nrm is not None:
                units.append(nrm)
        return units

    def proj_units(c):
        units = []

        def mk_proj(c, tbl, oc):
            def f():
                ps = pmisc.tile([128, CH], F32, name=f"psY{c}_{tbl}_{oc}", tag="pA")
                for p in range(NP):
                    _mm_kind(("proj", CH))
                    nc.tensor.matmul(
                        ps[:],
                        ot_sb[(c, p)][:, tbl * 128 : (tbl + 1) * 128],
                        wp_sb[p][:, oc * CH : (oc + 1) * CH],
                        start=(p == 0),
                        stop=(p == NP - 1),
                    )
                ye = yepool.tile([128, CH], DT_O, name=f"ye{c}_{tbl}_{oc}", tag="ye")
                nc.vector.tensor_copy(ye[:], ps[:])
                nc.sync.dma_start(
                    y[c * CH + tbl * 128 : c * CH + (tbl + 1) * 128,
                      oc * CH : (oc + 1) * CH],
                    ye[:],
                )

            return f

        for tbl in range(4):
            for oc in range(C // CH):
                units.append(mk_proj(c, tbl, oc))
        return units

    def emit_interleaved(primary, secondary):
        if not primary:
            for u in secondary:
                u()
            return
        ns, npri = len(secondary), len(primary)
        si = 0
        for i, u in enumerate(primary):
            u()
            want = (i + 1) * ns // npri
            while si < want:
                secondary[si]()
                si += 1

    # ---- emission ----
    warmup_pe()
    load_first_chunk()
    for u in stage_a_units(0):
        u()
    load_consts_late()
    for c in range(NCH):
        fill = []
        if c + 1 < NCH:
            fill += load_chunk_inputs(c + 1)
            fill += stage_a_units(c + 1)
        if c == NCH - 1:
            for cc in range(NCH - 1):
                fill += proj_units(cc)
        emit_interleaved(stage_b_units(c), fill)
    for u in proj_units(NCH - 1):
        u()


def build_nc():
    nc = bacc.Bacc("TRN2", target_bir_lowering=False, debug=False)
    xt4 = nc.declare_dram_parameter("xt4", [NCH, KT, 128, CH], DT_X, isOutput=False)
    wqk = nc.declare_dram_parameter("wqk", [KT, 128, 1024], DT_X, isOutput=False)
    wv = nc.declare_dram_parameter("wv", [KT, 128, 512], DT_X, isOutput=False)
    wp = nc.declare_dram_parameter("wp", [NP, 128, C], DT_O, isOutput=False)
    cs4 = nc.declare_dram_parameter("cs4", [NCH, 128, CH], DT_K, isOutput=False)
    sn4 = nc.declare_dram_parameter("sn4", [NCH, 128, CH], DT_K, isOutput=False)
    wedge = nc.declare_dram_parameter("wedge", [128, 128], DT_PV, isOutput=False)
    yout = nc.declare_dram_parameter("y", [T, C], BF16, isOutput=True)

    with tile.TileContext(nc) as tc:
        with ExitStack() as ctx:
            attn_body(
                ctx, tc, (yout[:],),
                (xt4[:], wqk[:], wv[:], wp[:], cs4[:], sn4[:], wedge[:]),
            )
    nc.compile()
    return nc


# ---------------- host side ----------------


def _rope_tables_np():
    inv_freq = 1.0 / (ROPE_BASE ** (np.arange(0, D, 2, dtype=np.float64) / D))
    t = np.arange(T, dtype=np.float64)
    freqs = np.outer(t, inv_freq)  # [T, 32]
    emb = np.concatenate([freqs, freqs], axis=-1)  # [T, 64]
    return np.cos(emb), np.sin(emb)  # [T, 64] each


def _host_tables():
    cos, sin = _rope_tables_np()  # [T, 64]
    d_of_r = np.arange(128) % 64
    cs = cos[:, d_of_r].T.astype(np.float32)  # [128, T]
    sn_abs = sin[:, d_of_r].T
    sign = np.where((d_of_r % 64) < 32, -1.0, 1.0)[:, None]
    sn = (sn_abs * sign).astype(np.float32)  # [128, T]
    np_k = _np_dt(DT_K)
    cs4 = np.ascontiguousarray(cs.reshape(128, NCH, CH).transpose(1, 0, 2)).astype(np_k)
    sn4 = np.ascontiguousarray(sn.reshape(128, NCH, CH).transpose(1, 0, 2)).astype(np_k)

    jj = np.arange(128)[:, None]
    ii = np.arange(128)[None, :]
    wedge = (jj <= ii).astype(np.float64)
    return cs4, sn4, wedge


def make_core_inputs(x, Wqkv, Wproj, core):
    b, g = core // 2, core % 2
    np_x = _np_dt(DT_X)
    np_pv = _np_dt(DT_PV)
    np_o = _np_dt(DT_O)

    xT = np.ascontiguousarray(x[b].T)  # [C, T]
    xt4 = np.ascontiguousarray(
        xT.reshape(KT, 128, NCH, CH).transpose(2, 0, 1, 3)
    ).astype(np_x)

    Wq = Wqkv[g * 512 : (g + 1) * 512]
    Wk = Wqkv[C + g * 512 : C + (g + 1) * 512]
    Wv = Wqkv[2 * C + g * 512 : 2 * C + (g + 1) * 512]
    wqkT = np.vstack([Wq, Wk]).T  # [C, 1024]
    wqk = np.ascontiguousarray(wqkT.reshape(KT, 128, 1024)).astype(np_x)
    wvT = Wv.T  # [C, 512]
    wv = np.ascontiguousarray(wvT.reshape(KT, 128, 512)).astype(np_x)
    wpT = Wproj[:, g * 512 : (g + 1) * 512].T  # [512, C]
    wp = np.ascontiguousarray(wpT.reshape(NP, 128, C)).astype(np_o)

    cs4, sn4, wedge = _host_tables()
    return {
        "xt4": xt4,
        "wqk": wqk,
        "wv": wv,
        "wp": wp,
        "cs4": cs4,
        "sn4": sn4,
        "wedge": wedge.astype(np_pv),
    }


LAST_RESULTS = None
_NC_CACHE = None


def kernel(x, Wqkv, Wproj):
    global LAST_RESULTS, _NC_CACHE
    from concourse.bass_utils import run_bass_kernel_spmd

    x = np.asarray(x, dtype=np.float32)
    Wqkv = np.asarray(Wqkv, dtype=np.float32)
    Wproj = np.asarray(Wproj, dtype=np.float32)

    if _NC_CACHE is None:
        _NC_CACHE = build_nc()
    nc = _NC_CACHE
    in_maps = [make_core_inputs(x, Wqkv, Wproj, core) for core in range(NCORES)]
    res = run_bass_kernel_spmd(nc, in_maps, list(range(NCORES)))
    LAST_RESULTS = res

    out = np.empty((B, T, C), dtype=np.float32)
    for b in range(B):
        out[b] = (res.results[2 * b]["y"].astype(np.float32)
                  + res.results[2 * b + 1]["y"].astype(np.float32))
    return out


# revision 9
# speedup vs baseline: 1.0025x; 1.0016x over previous
"""Causal self-attention (RoPE) Trainium2 Bass kernel, v2.

Problem: B=4, T=2048, C=1024, H=16 heads, D=64, fp32 I/O.
Sharding: 8 cores = 4 (batch) x 2 (head-group TP). Each core computes
qkv/attention/proj for 1 batch and 8 heads, producing a partial
projection output; the host sums the two TP partials per batch.

Changes over the original baseline (326us -> ~301us on hardware):
- RoPE: ScalarE evacuates the qkv PSUM tile to bf16 (off the DVE critical
  path), sbuf->sbuf DMA performs the cross-partition rotate-half swap, and
  three fast SBUF tensor_tensor ops finish rot = q*cos + swap*sin.
- Causal ioff shrink applied to chunk 0 as well (was full-width).
- One exp ACTIVATE per head per score unit when io0==0 (the short
  unwritten PSUM gap between halves is never consumed); two calls
  otherwise.
- Causal masks multiply only the 128-wide diagonal wedges with a shared
  [128,128] lower-triangle constant (vs full-tile masks).
- Stage-B PV matmuls lag one unit behind QK+exp (software pipeline), so
  PV never stalls the PE queue waiting on the exp.
- ~5.5us of zero-data warmup matmuls overlap the initial DMA window so
  the PE HAM clock-gate is at 2.4 GHz when real work issues.
- y output in bf16 (halves the output DMA; host sums partials in fp32).
Notes from tuning: QK matmuls stay in 64-row mode with K=64 -- the
zero-padded full-array variant tripped the package power limit (P0,
whole chip drops to 2.0 GHz). PE issue spacing is gated by the
predecessor matmul's free dim (~N/2.4GHz), so the kernel is within ~20%
of the PE stream floor for this algorithm.
"""

import numpy as np
from contextlib import ExitStack

import concourse.bacc as bacc
import concourse.bass as bass
import concourse.mybir as mybir
import concourse.tile as tile

# ---------------- constants ----------------
B = 4
T = 2048
C = 1024
H = 16
D = 64
L = 8  # local heads per core
NCORES = 8
ROPE_BASE = 10000.0

CH = 512  # t-chunk size
NCH = T // CH  # 4 chunks
KT = C // 128  # 8 contraction tiles
NP = L // 2  # 4 head-pair tiles
SCALE = 1.0 / np.sqrt(D)

F32 = mybir.dt.float32
BF16 = mybir.dt.bfloat16

DT_X = BF16
DT_K = BF16
DT_PV = BF16
DT_O = BF16


def _np_dt(dt):
    return mybir.dt.np(dt)


# ---------------- device kernel ----------------


def attn_body(ctx: ExitStack, tc: tile.TileContext, outs, ins):
    """outs = (y [T, C] f32,); ins = (xt4, wqk, wv, wp, cs4, sn4, perm, wedge)."""
    nc = tc.nc
    (y,) = outs if isinstance(outs, (tuple, list)) else (outs,)
    xt4, wqk, wv, wp, cs4, sn4, wedge = ins

    consts = ctx.enter_context(tc.tile_pool(name="consts", bufs=1))
    xpool = ctx.enter_context(tc.tile_pool(name="xpool", bufs=16))
    cspool = ctx.enter_context(tc.tile_pool(name="cspool", bufs=4))
    qrpool = ctx.enter_context(tc.tile_pool(name="qrpool", bufs=8))
    rtmp = ctx.enter_context(tc.tile_pool(name="rtmp", bufs=6))
    ptpool = ctx.enter_context(tc.tile_pool(name="ptpool", bufs=6))
    otpool = ctx.enter_context(tc.tile_pool(name="otpool", bufs=16))
    yepool = ctx.enter_context(tc.tile_pool(name="yepool", bufs=3))
    lpool = ctx.enter_context(tc.tile_pool(name="lpool", bufs=3))
    pmisc = ctx.enter_context(tc.tile_pool(name="pmisc", bufs=2, space="PSUM"))
    pss_pool = ctx.enter_context(tc.tile_pool(name="pss", bufs=2, space="PSUM"))
    pso_pool = ctx.enter_context(tc.tile_pool(name="pso", bufs=2, space="PSUM"))

    # persistent tiles
    wqk_sb = [consts.tile([128, 2 * 512], DT_X, name=f"wqk{k}") for k in range(KT)]
    wv_sb = [consts.tile([128, 512], DT_X, name=f"wv{k}") for k in range(KT)]
    wp_sb = [consts.tile([128, C], DT_O, name=f"wp{p}") for p in range(NP)]
    wedge_sb = consts.tile([128, 128], DT_PV, name="wedge")
    k_rot = [consts.tile([128, T], DT_K, name=f"krot{p}") for p in range(NP)]
    v_sb = consts.tile([128, T // 128, L, 65], DT_PV, name="vsb")

    def warmup_pe():
        # ~5.5us of dummy matmuls on a zeroed tile so the PE HAM clock-gate
        # is warm (2.4 GHz) by the time real matmuls issue, overlapping the
        # initial DMA window.
        warm = consts.tile([128, CH], DT_X, name="warm")
        nc.vector.memset(warm[:], 0.0)
        wps = pmisc.tile([128, CH], F32, name="warmps", tag="pA")
        for i in range(26):
            nc.tensor.matmul(wps[:], warm[:, 0:128], warm[:],
                             start=(i == 0), stop=(i == 25))

    def load_first_chunk():
        xt_sb[0] = []
        for k in range(KT):
            nc.sync.dma_start(wqk_sb[k][:], wqk[k])
            xt = xpool.tile([128, CH], DT_X, name=f"xt0_{k}", tag="xt")
            nc.sync.dma_start(xt[:], xt4[0, k])
            xt_sb[0].append(xt)
        cs_sb[0] = cspool.tile([128, CH], DT_K, name="cs0", tag="cs")
        sn_sb[0] = cspool.tile([128, CH], DT_K, name="sn0", tag="sn")
        nc.sync.dma_start(cs_sb[0][:], cs4[0])
        nc.sync.dma_start(sn_sb[0][:], sn4[0])
        for k in range(KT):
            nc.sync.dma_start(wv_sb[k][:], wv[k])
        nc.vector.memset(v_sb[:, :, :, 64:65], 1.0)

    def load_consts_late():
        nc.sync.dma_start(wedge_sb[:], wedge[:])
        for p in range(NP):
            nc.sync.dma_start(wp_sb[p][:], wp[p])

    # per-chunk transient state
    xt_sb = {}
    q_rot = {}
    cs_sb = {}
    sn_sb = {}
    ot_sb = {}

    def load_chunk_inputs(c):
        def f():
            cs_sb[c] = cspool.tile([128, CH], DT_K, name=f"cs{c}", tag="cs")
            sn_sb[c] = cspool.tile([128, CH], DT_K, name=f"sn{c}", tag="sn")
            nc.sync.dma_start(cs_sb[c][:], cs4[c])
            nc.sync.dma_start(sn_sb[c][:], sn4[c])
            xt_sb[c] = []
            for k in range(KT):
                xt = xpool.tile([128, CH], DT_X, name=f"xt{c}_{k}", tag="xt")
                nc.sync.dma_start(xt[:], xt4[c, k])
                xt_sb[c].append(xt)

        return [f]

    def stage_a_units(c):
        """12 units: 8 q/k feature tiles + 4 v t-blocks for chunk c.

        RoPE per q/k tile: ACT evacuates PSUM to bf16 (freeing the PSUM slot
        off the DVE path), DMA performs the cross-partition rotate-half swap,
        and three fast SBUF tensor_tensor ops finish rot = q*cos + swap*sin.
        """

        def mk_qk(c, jt):
            def f():
                ps = pmisc.tile([128, CH], F32, name=f"psA{c}_{jt}", tag="pA")
                for k in range(KT):
                    nc.tensor.matmul(
                        ps[:],
                        wqk_sb[k][:, jt * 128 : (jt + 1) * 128],
                        xt_sb[c][k][:],
                        start=(k == 0),
                        stop=(k == KT - 1),
                    )
                q_sb = rtmp.tile([128, CH], DT_K, name=f"qsb{c}_{jt}", tag="qsb")
                nc.scalar.activation(
                    q_sb[:], ps[:], mybir.ActivationFunctionType.Copy
                )
                qsw = rtmp.tile([128, CH], DT_K, name=f"qsw{c}_{jt}", tag="qsw")
                for blk in range(2):
                    b0 = blk * 64
                    nc.sync.dma_start(
                        qsw[b0 : b0 + 32, :], q_sb[b0 + 32 : b0 + 64, :]
                    )
                    nc.sync.dma_start(
                        qsw[b0 + 32 : b0 + 64, :], q_sb[b0 : b0 + 32, :]
                    )
                qtmp = rtmp.tile([128, CH], DT_K, name=f"qtmp{c}_{jt}", tag="qtmp")
                nc.vector.tensor_tensor(
                    out=qtmp[:], in0=qsw[:], in1=sn_sb[c][:],
                    op=mybir.AluOpType.mult,
                )
                qraw = rtmp.tile([128, CH], DT_K, name=f"qraw{c}_{jt}", tag="qraw")
                nc.vector.tensor_tensor(
                    out=qraw[:], in0=q_sb[:], in1=cs_sb[c][:],
                    op=mybir.AluOpType.mult,
                )
                if jt < NP:
                    dst_t = qrpool.tile([128, CH], DT_K, name=f"qrot{c}_{jt}", tag="qr")
                    q_rot[(c, jt)] = dst_t
                    out_ap = dst_t[:]
                else:
                    out_ap = k_rot[jt - NP][:, c * CH : (c + 1) * CH]
                nc.vector.tensor_tensor(
                    out=out_ap, in0=qraw[:], in1=qtmp[:], op=mybir.AluOpType.add
                )

            return f

        def mk_v(c, tbl):
            tb = c * 4 + tbl

            def f():
                ps = pmisc.tile([128, CH], F32, name=f"psV{c}_{tbl}", tag="pA")
                for k in range(KT):
                    nc.tensor.matmul(
                        ps[:],
                        xt_sb[c][k][:, tbl * 128 : (tbl + 1) * 128],
                        wv_sb[k][:],
                        start=(k == 0),
                        stop=(k == KT - 1),
                    )
                nc.vector.tensor_copy(
                    v_sb[:, tb, :, 0:64],
                    ps[:].rearrange("p (h d) -> p h d", h=L),
                )

            return f

        units = [mk_qk(c, jt) for jt in range(2 * NP)]
        units += [mk_v(c, tbl) for tbl in range(4)]
        return units

    def stage_b_units(c):
        """QK+exp+mask / PV micro-units with PV lagging one unit."""

        def _ioff(jb, jp):
            return (jb - 4 * c) * 128 if jp >= 2 * c else 0

        pss_unit = {}
        pt_unit = {}
        pso_unit = {}

        def mk_qk_exp(p, jp):
            def f():
                ha, hb = 2 * p, 2 * p + 1
                qt = q_rot[(c, p)]
                kt_ = k_rot[p]
                for h in (ha, hb):
                    pss_unit[(h, jp)] = pss_pool.tile(
                        [128, 2 * CH], F32, name=f"pss{c}_{p}_{jp}_{h}", tag="pss"
                    )
                # 64-row-mode score matmuls (K=64). Half-array activity keeps
                # 8-core package power below the P0 downclock threshold --
                # full-array variants measurably drop the whole chip to 2 GHz.
                for half in range(2):
                    jb = 2 * jp + half
                    io = _ioff(jb, jp)
                    for h, rb in ((ha, 0), (hb, 64)):
                        nc.tensor.matmul(
                            pss_unit[(h, jp)][:, half * CH + io : (half + 1) * CH],
                            kt_[rb : rb + 64, jb * 128 : (jb + 1) * 128],
                            qt[rb : rb + 64, io:],
                            start=True,
                            stop=True,
                        )
                io0 = _ioff(2 * jp, jp)
                io1 = _ioff(2 * jp + 1, jp)
                for h in (ha, hb):
                    pt = ptpool.tile(
                        [128, 2 * CH], DT_PV, name=f"pt{c}_{p}_{jp}_{h}", tag="pt"
                    )
                    pt_unit[(h, jp)] = pt
                    if io0 == 0:
                        # one call covers [0, 2CH); the short unwritten gap
                        # before CH+io1 yields garbage that is never consumed
                        nc.scalar.activation(
                            pt[:],
                            pss_unit[(h, jp)][:],
                            mybir.ActivationFunctionType.Exp,
                            scale=float(SCALE),
                        )
                    else:
                        for lo, hi in ((io0, CH), (CH + io1, 2 * CH)):
                            nc.scalar.activation(
                                pt[:, lo:hi],
                                pss_unit[(h, jp)][:, lo:hi],
                                mybir.ActivationFunctionType.Exp,
                                scale=float(SCALE),
                            )
                    if jp >= 2 * c:  # diagonal: mask the two 128-wide wedges
                        for half in range(2):
                            jb = 2 * jp + half
                            io = _ioff(jb, jp)
                            lo = half * CH + io
                            nc.vector.tensor_tensor(
                                out=pt[:, lo : lo + 128],
                                in0=pt[:, lo : lo + 128],
                                in1=wedge_sb[:],
                                op=mybir.AluOpType.mult,
                            )

            return f

        def mk_pv(p, jp):
            def f():
                ha, hb = 2 * p, 2 * p + 1
                if jp == 0:
                    for h in (ha, hb):
                        pso_unit[h] = pso_pool.tile(
                            [65, CH], F32, name=f"pso{c}_{h}", tag="pso"
                        )
                for h in (ha, hb):
                    pt = pt_unit.pop((h, jp))
                    pso = pso_unit[h]
                    for half in range(2):
                        jb = 2 * jp + half
                        io = _ioff(jb, jp)
                        nc.tensor.matmul(
                            pso[:, io:],
                            v_sb[:, jb, h, 0:65],
                            pt[:, half * CH + io : (half + 1) * CH],
                            start=(jp == 0 and half == 0),
                            stop=(jp == 2 * c + 1 and half == 1),
                        )

            return f

        def mk_norm(p):
            def f():
                ot = ot_sb[(c, p)] = otpool.tile(
                    [128, CH], DT_O, name=f"ot{c}_{p}", tag="ot"
                )
                for idx, h in enumerate((2 * p, 2 * p + 1)):
                    pso = pso_unit[h]
                    # reciprocal_approx_fast cannot read PSUM (custom DVE op);
                    # evacuate the denominator row via ScalarE first.
                    lsb = lpool.tile([1, CH], F32, name=f"lsb{c}_{h}", tag="lsb")
                    nc.vector.tensor_copy(lsb[:], pso[64:65, :])
                    linv = lpool.tile([1, CH], F32, name=f"linv{c}_{h}", tag="linv")
                    nc.vector.reciprocal_approx_fast(linv[:], lsb[:])
                    lb = lpool.tile([64, CH], F32, name=f"lb{c}_{h}", tag="lb")
                    nc.gpsimd.partition_broadcast(lb[:], linv[:])
                    nc.vector.tensor_tensor(
                        out=ot[idx * 64 : (idx + 1) * 64, :],
                        in0=pso[0:64, :],
                        in1=lb[:],
                        op=mybir.AluOpType.mult,
                    )

            return f

        units = []
        pending = []  # (pv_unit, norm_or_None)
        for p in range(NP):
            njp = 2 * c + 2
            for jp in range(njp):
                units.append(mk_qk_exp(p, jp))
                pending.append((mk_pv(p, jp), mk_norm(p) if jp == njp - 1 else None))
                if len(pending) > 1:
                    pv, nrm = pending.pop(0)
                    units.append(pv)
                    if nrm is not None:
                        units.append(nrm)
        while pending:
            pv, nrm = pending.pop(0)
            units.append(pv)
            if nrm is not None:
                units.append(nrm)
        return units

    def proj_units(c):
        units = []

        def mk_proj(c, tbl, oc):
            def f():
                ps = pmisc.tile([128, CH], F32, name=f"psY{c}_{tbl}_{oc}", tag="pA")
                for p in range(NP):
                    nc.tensor.matmul(
                        ps[:],
                        ot_sb[(c, p)][:, tbl * 128 : (tbl + 1) * 128],
                        wp_sb[p][:, oc * CH : (oc + 1) * CH],
                        start=(p == 0),
                        stop=(p == NP - 1),
                    )
                ye = yepool.tile([128, CH], DT_O, name=f"ye{c}_{tbl}_{oc}", tag="ye")
                nc.vector.tensor_copy(ye[:], ps[:])
                nc.sync.dma_start(
                    y[c * CH + tbl * 128 : c * CH + (tbl + 1) * 128,
                      oc * CH : (oc + 1) * CH],
                    ye[:],
                )

            return f

        for tbl in range(4):
            for oc in range(C // CH):
                units.append(mk_proj(c, tbl, oc))
        return units

    def emit_interleaved(primary, secondary):
        if not primary:
            for u in secondary:
                u()
            return
        ns, npri = len(secondary), len(primary)
        si = 0
        for i, u in enumerate(primary):
            u()
            want = (i + 1) * ns // npri
            while si < want:
                secondary[si]()
                si += 1

    # ---- emission ----
    warmup_pe()
    load_first_chunk()
    for u in stage_a_units(0):
        u()
    load_consts_late()
    for c in range(NCH):
        fill = []
        if c + 1 < NCH:
            fill += load_chunk_inputs(c + 1)
            fill += stage_a_units(c + 1)
        if c == NCH - 1:
            for cc in range(NCH - 1):
                fill += proj_units(cc)
        emit_interleaved(stage_b_units(c), fill)
    for u in proj_units(NCH - 1):
        u()


def build_nc():
    nc = bacc.Bacc("TRN2", target_bir_lowering=False, debug=False)
    xt4 = nc.declare_dram_parameter("xt4", [NCH, KT, 128, CH], DT_X, isOutput=False)
    wqk = nc.declare_dram_parameter("wqk", [KT, 128, 1024], DT_X, isOutput=False)
    wv = nc.declare_dram_parameter("wv", [KT, 128, 512], DT_X, isOutput=False)
    wp = nc.declare_dram_parameter("wp", [NP, 128, C], DT_O, isOutput=False)
    cs4 = nc.declare_dram_parameter("cs4", [NCH, 128, CH], DT_K, isOutput=False)
    sn4 = nc.declare_dram_parameter("sn4", [NCH, 128, CH], DT_K, isOutput=False)
    wedge = nc.declare_dram_parameter("wedge", [128, 128], DT_PV, isOutput=False)
    yout = nc.declare_dram_parameter("y", [T, C], BF16, isOutput=True)

    with tile.TileContext(nc) as tc:
        with ExitStack() as ctx:
            attn_body(
                ctx, tc, (yout[:],),
                (xt4[:], wqk[:], wv[:], wp[:], cs4[:], sn4[:], wedge[:]),
            )
    nc.compile()
    return nc


# ---------------- host side ----------------


def _rope_tables_np():
    inv_freq = 1.0 / (ROPE_BASE ** (np.arange(0, D, 2, dtype=np.float64) / D))
    t = np.arange(T, dtype=np.float64)
    freqs = np.outer(t, inv_freq)  # [T, 32]
    emb = np.concatenate([freqs, freqs], axis=-1)  # [T, 64]
    return np.cos(emb), np.sin(emb)  # [T, 64] each


def _host_tables():
    cos, sin = _rope_tables_np()  # [T, 64]
    d_of_r = np.arange(128) % 64
    cs = cos[:, d_of_r].T.astype(np.float32)  # [128, T]
    sn_abs = sin[:, d_of_r].T
    sign = np.where((d_of_r % 64) < 32, -1.0, 1.0)[:, None]
    sn = (sn_abs * sign).astype(np.float32)  # [128, T]
    np_k = _np_dt(DT_K)
    cs4 = np.ascontiguousarray(cs.reshape(128, NCH, CH).transpose(1, 0, 2)).astype(np_k)
    sn4 = np.ascontiguousarray(sn.reshape(128, NCH, CH).transpose(1, 0, 2)).astype(np_k)

    jj = np.arange(128)[:, None]
    ii = np.arange(128)[None, :]
    wedge = (jj <= ii).astype(np.float64)
    return cs4, sn4, wedge


def make_core_inputs(x, Wqkv, Wproj, core):
    b, g = core // 2, core % 2
    np_x = _np_dt(DT_X)
    np_pv = _np_dt(DT_PV)
    np_o = _np_dt(DT_O)

    xT = np.ascontiguousarray(x[b].T)  # [C, T]
    xt4 = np.ascontiguousarray(
        xT.reshape(KT, 128, NCH, CH).transpose(2, 0, 1, 3)
    ).astype(np_x)

    Wq = Wqkv[g * 512 : (g + 1) * 512]
    Wk = Wqkv[C + g * 512 : C + (g + 1) * 512]
    Wv = Wqkv[2 * C + g * 512 : 2 * C + (g + 1) * 512]
    wqkT = np.vstack([Wq, Wk]).T  # [C, 1024]
    wqk = np.ascontiguousarray(wqkT.reshape(KT, 128, 1024)).astype(np_x)
    wvT = Wv.T  # [C, 512]
    wv = np.ascontiguousarray(wvT.reshape(KT, 128, 512)).astype(np_x)
    wpT = Wproj[:, g * 512 : (g + 1) * 512].T  # [512, C]
    wp = np.ascontiguousarray(wpT.reshape(NP, 128, C)).astype(np_o)

    cs4, sn4, wedge = _host_tables()
    return {
        "xt4": xt4,
        "wqk": wqk,
        "wv": wv,
        "wp": wp,
        "cs4": cs4,
        "sn4": sn4,
        "wedge": wedge.astype(np_pv),
    }


LAST_RESULTS = None
_NC_CACHE = None


def kernel(x, Wqkv, Wproj):
    global LAST_RESULTS, _NC_CACHE
    from concourse.bass_utils import run_bass_kernel_spmd

    x = np.asarray(x, dtype=np.float32)
    Wqkv = np.asarray(Wqkv, dtype=np.float32)
    Wproj = np.asarray(Wproj, dtype=np.float32)

    if _NC_CACHE is None:
        _NC_CACHE = build_nc()
    nc = _NC_CACHE
    in_maps = [make_core_inputs(x, Wqkv, Wproj, core) for core in range(NCORES)]
    res = run_bass_kernel_spmd(nc, in_maps, list(range(NCORES)))
    LAST_RESULTS = res

    out = np.empty((B, T, C), dtype=np.float32)
    for b in range(B):
        out[b] = (res.results[2 * b]["y"].astype(np.float32)
                  + res.results[2 * b + 1]["y"].astype(np.float32))
    return out
